# revision 1
# baseline (speedup 1.0000x reference)
"""Trainium2 Bass kernel for nn_ContextEmbedding (embedding lookup + masked MLP branches).

Strategy (data-parallel over 8 cores, batch-sharded):
  out[r, :] = onehot8(tok[r]) @ special_table            (~all rows; exact, incl. zeros)
            + [tok[r]==CLS]  * relu(LN(x3 @ cls_w + b))  (~1/76 of rows)
            + [tok[r]==CTX]  * relu(LN(x16 @ ctx_w + b)) (~1/76 of rows)

Dense pass: one bf16 matmul per 128-row chunk (one-hot is exact in bf16; the fp32
table is split into bf16 hi+lo halves stacked along K so a single K=16 matmul
reconstructs ~fp32 precision), then PSUM->SBUF copy and a contiguous DMA out.

Sparse fixup: the CLS/CTX rows are compacted on host, the branch MLP+LN+ReLU runs
on a handful of 128-row tiles, the per-row results (plus the token's table row)
are scattered back over the dense output via indirect DMA (padding lanes carry
out-of-bounds indices and are skipped).
"""

import os
import time
import numpy as np
import ml_dtypes

import concourse.bass as bass
import concourse.bacc as bacc
import concourse.mybir as mybir
from concourse.tile import TileContext
from concourse.bass_utils import run_bass_kernel_spmd

N_CORES = 8
B, S, D = 256, 512, 256
NUM_CONTEXT = 16
NUM_SPECIAL = 8
SPECIAL_OFFSET = 68  # 52 cards + 16 bet bins
CLS_TOK = SPECIAL_OFFSET + 0
CTX_TOK = SPECIAL_OFFSET + 1
LN_EPS = 1e-5
P = 128
R = (B * S) // N_CORES       # rows per core
CHUNKS = R // P
OOB_PAD = 1 << 20

_LAST = {}


def _branch_host(W, bvec, g, b_ln):
    """Host-side prep of one MLP branch: center the linear layer so the LN mean
    subtraction folds into the weights, and (when LN gamma is uniform) fold
    gamma in too. Returns the rhs matrix for the device matmul plus the scale
    constant for the sqrt(var+eps) activation."""
    W64 = np.asarray(W, np.float64)
    b64 = np.asarray(bvec, np.float64)
    g64 = np.asarray(g, np.float64)
    bln64 = np.asarray(b_ln, np.float64)
    wm = W64.mean(axis=1, keepdims=True)
    bm = b64.mean()
    Wc = W64 - wm
    bc = b64 - bm
    g_uniform = bool(np.all(g64 == g64.flat[0]))
    use_bln = bool(np.any(bln64 != 0.0))
    if g_uniform and not use_bln:
        gv = float(g64.flat[0])
        if gv == 0.0:
            return dict(mode="zero")
        rhs = np.concatenate([Wc * gv, (bc * gv)[None, :]], axis=0)  # [K+1, D]
        return dict(mode="fast", rhs=rhs.astype(np.float32),
                    sqrt_scale=float(1.0 / (D * gv * gv)))
    # general: rhs = [hc block | hg block]; hc drives the variance, hg the output
    Wg = Wc * g64[None, :]
    bg = bc * g64
    rhs = np.concatenate(
        [np.concatenate([Wc, bc[None, :]], axis=0),
         np.concatenate([Wg, bg[None, :]], axis=0)], axis=1)  # [K+1, 2D]
    return dict(mode="general", rhs=rhs.astype(np.float32),
                sqrt_scale=float(1.0 / D), use_bln=use_bln,
                bln_rep=np.tile(bln64.astype(np.float32)[None, :], (P, 1)))


def _compact(tok_flat, x_flat, token_value, k_feats):
    """Compact the rows with token==token_value, per core. Returns per-core
    transposed (bias-augmented) feature tiles and int32 scatter indices padded
    with OOB_PAD (skipped by the indirect DMA bounds check)."""
    per_core = [np.nonzero(tok_flat[c * R:(c + 1) * R] == token_value)[0]
                for c in range(N_CORES)]
    max_n = max(len(rows) for rows in per_core)
    if max_n == 0:
        return None
    T = (max_n + P - 1) // P
    npad = T * P
    xgts, idxs = [], []
    for c in range(N_CORES):
        rows = per_core[c]
        n = len(rows)
        xg = np.zeros((k_feats + 1, npad), np.float32)
        xg[k_feats, :] = 1.0  # bias row
        if n:
            xs = x_flat[c * R + rows][:, :k_feats]
            xg[:k_feats, :n] = np.ascontiguousarray(xs.T.astype(np.float32))
        idx = np.full((P, T), OOB_PAD, np.int32)
        if n:
            jj = np.arange(n)
            idx[jj % P, jj // P] = rows.astype(np.int32)
        xgts.append(np.ascontiguousarray(xg))
        idxs.append(np.ascontiguousarray(idx))
    return dict(T=T, xgt=xgts, idx=idxs)


def _build(meta):
    nc = bacc.Bacc(None)
    f32 = mybir.dt.float32
    bf16 = mybir.dt.bfloat16
    i32 = mybir.dt.int32
    Relu = mybir.ActivationFunctionType.Relu
    Sqrt = mybir.ActivationFunctionType.Sqrt
    Square = mybir.ActivationFunctionType.Square

    oh_d = nc.dram_tensor("oh", [2 * NUM_SPECIAL, R], bf16, kind="ExternalInput")
    rhs16_d = nc.dram_tensor("rhs16", [2 * NUM_SPECIAL, D], bf16, kind="ExternalInput")
    out_d = nc.dram_tensor("out", [R, D], f32, kind="ExternalOutput")

    br_handles = {}
    for name, br in meta["branches"].items():
        if br is None:
            continue
        K1, ND = br["host"]["rhs"].shape
        T = br["comp"]["T"]
        h = dict(
            xgt=nc.dram_tensor(f"xgt_{name}", [K1, T * P], f32, kind="ExternalInput"),
            w=nc.dram_tensor(f"w_{name}", [K1, ND], f32, kind="ExternalInput"),
            idx=nc.dram_tensor(f"idx_{name}", [P, T], i32, kind="ExternalInput"),
            tbl=nc.dram_tensor(f"tbl_{name}", [P, D], f32, kind="ExternalInput"),
        )
        if br["host"].get("use_bln"):
            h["bln"] = nc.dram_tensor(f"bln_{name}", [P, D], f32, kind="ExternalInput")
        br_handles[name] = h

    with TileContext(nc) as tc:
        with (
            tc.tile_pool(name="const", bufs=1) as cpool,
            tc.tile_pool(name="dense_in", bufs=6) as inpool,
            tc.tile_pool(name="dense_out", bufs=8) as outpool,
            tc.tile_pool(name="fix", bufs=max(4, 2 * meta["t_max"])) as fixpool,
        ):
            rhs16_sb = cpool.tile([2 * NUM_SPECIAL, D], bf16, tag="rhs16")
            nc.sync.dma_start(out=rhs16_sb[:], in_=rhs16_d[:])
            eps_sb = cpool.tile([P, 1], f32, tag="eps")
            nc.vector.memset(eps_sb[:], LN_EPS)

            br_sb = {}
            for name, h in br_handles.items():
                br = meta["branches"][name]
                K1, ND = br["host"]["rhs"].shape
                T = br["comp"]["T"]
                d = {}
                d["xgt"] = cpool.tile([K1, T * P], f32, tag=f"xgt_{name}", name=f"xgt_{name}_sb")
                nc.sync.dma_start(out=d["xgt"][:], in_=h["xgt"][:])
                d["w"] = cpool.tile([K1, ND], f32, tag=f"w_{name}", name=f"w_{name}_sb")
                nc.sync.dma_start(out=d["w"][:], in_=h["w"][:])
                d["idx"] = cpool.tile([P, T], i32, tag=f"idx_{name}", name=f"idx_{name}_sb")
                nc.sync.dma_start(out=d["idx"][:], in_=h["idx"][:])
                d["tbl"] = cpool.tile([P, D], f32, tag=f"tbl_{name}", name=f"tbl_{name}_sb")
                nc.sync.dma_start(out=d["tbl"][:], in_=h["tbl"][:])
                if "bln" in h:
                    d["bln"] = cpool.tile([P, D], f32, tag=f"bln_{name}", name=f"bln_{name}_sb")
                    nc.sync.dma_start(out=d["bln"][:], in_=h["bln"][:])
                br_sb[name] = d

            # ---- dense pass ----
            # G row-chunks per group: one SP-ring load, G matmuls, one big ACT
            # copy, one ACT-ring store. Fresh tiles every group + same-engine
            # (ACT) copy->store keep every HWDGE DMA at <=1 sync wait (the
            # hardware instruction only fits one wait + its completion update).
            G = 4
            NG = CHUNKS // G
            out_v = out_d[:].rearrange("(g q p) d -> g p q d", p=P, q=G)
            with tc.tile_pool(name="psd", bufs=4, space="PSUM") as psd:
                for g in range(NG):
                    oh_t = inpool.tile([2 * NUM_SPECIAL, G * P], bf16, tag="oh")
                    nc.sync.dma_start(out=oh_t[:], in_=oh_d[:, g * G * P:(g + 1) * G * P])
                    ps = psd.tile([P, G * D], f32, tag="dps")
                    for q in range(G):
                        nc.tensor.matmul(out=ps[:, q * D:(q + 1) * D],
                                         lhsT=oh_t[:, q * P:(q + 1) * P],
                                         rhs=rhs16_sb[:], start=True, stop=True)
                    ot = outpool.tile([P, G * D], f32, tag="dout")
                    if g % 2 == 0:
                        nc.vector.tensor_copy(out=ot[:], in_=ps[:])
                        nc.sync.dma_start(out=out_v[g], in_=ot[:])
                    else:
                        nc.scalar.copy(out=ot[:], in_=ps[:])
                        nc.scalar.dma_start(out=out_v[g], in_=ot[:])

            # ---- sparse fixup (both branches interleaved, func-major to
            # minimize ACT LUT-table swaps) ----
            pairs = []
            if not os.environ.get("KERNEL_DENSE_ONLY"):
                for name in ("ctx", "cls"):
                    if name in br_sb:
                        pairs.extend((name, t) for t in range(meta["branches"][name]["comp"]["T"]))
            if pairs:
                with tc.tile_pool(name="psf", bufs=meta["t_max"], space="PSUM") as psf:
                    psA, ss, sd, rstd, rr = {}, {}, {}, {}, {}
                    for name, t in pairs:
                        br = meta["branches"][name]
                        ND = br["host"]["rhs"].shape[1]
                        s = br_sb[name]
                        psA[(name, t)] = psf.tile([P, ND], f32, tag=f"psA_{name}", name=f"psA_{name}_{t}")
                        nc.tensor.matmul(out=psA[(name, t)][:],
                                         lhsT=s["xgt"][:, t * P:(t + 1) * P],
                                         rhs=s["w"][:], start=True, stop=True)
                    for name, t in pairs:
                        sq = fixpool.tile([P, D], f32, tag="sq")
                        ss[(name, t)] = fixpool.tile([P, 1], f32, tag="ss", name=f"ss_{name}_{t}")
                        nc.scalar.activation(out=sq[:], in_=psA[(name, t)][:, 0:D],
                                             func=Square, accum_out=ss[(name, t)][:])
                    for name, t in pairs:
                        sqs = meta["branches"][name]["host"]["sqrt_scale"]
                        sd[(name, t)] = fixpool.tile([P, 1], f32, tag="sd", name=f"sd_{name}_{t}")
                        nc.scalar.activation(out=sd[(name, t)][:], in_=ss[(name, t)][:],
                                             func=Sqrt, bias=eps_sb[:, 0:1], scale=sqs)
                    for name, t in pairs:
                        rstd[(name, t)] = fixpool.tile([P, 1], f32, tag="rstd", name=f"rstd_{name}_{t}")
                        nc.vector.reciprocal(out=rstd[(name, t)][:], in_=sd[(name, t)][:])
                    for name, t in pairs:
                        host = meta["branches"][name]["host"]
                        rr[(name, t)] = fixpool.tile([P, D], f32, tag="rr", name=f"rr_{name}_{t}")
                        if host["mode"] == "fast":
                            nc.scalar.activation(out=rr[(name, t)][:],
                                                 in_=psA[(name, t)][:, 0:D],
                                                 func=Relu, scale=rstd[(name, t)][:, 0:1])
                        else:
                            pre = fixpool.tile([P, D], f32, tag="pre")
                            nc.vector.tensor_scalar_mul(out=pre[:],
                                                        in0=psA[(name, t)][:, D:2 * D],
                                                        scalar1=rstd[(name, t)][:, 0:1])
                            if host.get("use_bln"):
                                nc.vector.tensor_add(out=pre[:], in0=pre[:],
                                                     in1=br_sb[name]["bln"][:])
                            nc.scalar.activation(out=rr[(name, t)][:], in_=pre[:],
                                                 func=Relu)
                    for name, t in pairs:
                        s = br_sb[name]
                        fx = fixpool.tile([P, D], f32, tag="fx")
                        nc.vector.tensor_add(out=fx[:], in0=rr[(name, t)][:],
                                             in1=s["tbl"][:])
                        nc.gpsimd.indirect_dma_start(
                            out=out_d[:],
                            out_offset=bass.IndirectOffsetOnAxis(
                                ap=s["idx"][:, t:t + 1], axis=0),
                            in_=fx[:],
                            in_offset=None,
                            bounds_check=R - 1,
                            oob_is_err=False,
                        )
    nc.compile()
    return nc


def kernel(**inputs):
    tok = np.asarray(inputs["token_ids"]).reshape(-1).astype(np.int64)
    x = np.asarray(inputs["context_features"], np.float32).reshape(-1, NUM_CONTEXT)
    st = np.asarray(inputs["special_table"], np.float32)

    # one-hot over the 8 special ids, exact in bf16; stacked twice for the
    # hi/lo split of the fp32 table (single K=16 bf16 matmul ~ fp32 result)
    oh8 = (tok[None, :] == (SPECIAL_OFFSET + np.arange(NUM_SPECIAL))[:, None])
    oh16 = np.concatenate([oh8, oh8], axis=0).astype(ml_dtypes.bfloat16)
    t_hi = st.astype(ml_dtypes.bfloat16)
    t_lo = (st - t_hi.astype(np.float32)).astype(ml_dtypes.bfloat16)
    rhs16 = np.ascontiguousarray(np.concatenate([t_hi, t_lo], axis=0))

    branches = {}
    comp_cls = _compact(tok, x, CLS_TOK, 3)
    comp_ctx = _compact(tok, x, CTX_TOK, NUM_CONTEXT)
    host_cls = _branch_host(inputs["cls_w"], inputs["cls_b"],
                            inputs["cls_ln_g"], inputs["cls_ln_b"])
    host_ctx = _branch_host(inputs["ctx_w"], inputs["ctx_b"],
                            inputs["ctx_ln_g"], inputs["ctx_ln_b"])
    branches["cls"] = (dict(host=host_cls, comp=comp_cls)
                       if comp_cls is not None and host_cls["mode"] != "zero" else None)
    branches["ctx"] = (dict(host=host_ctx, comp=comp_ctx)
                       if comp_ctx is not None and host_ctx["mode"] != "zero" else None)
    t_max = max([br["comp"]["T"] for br in branches.values() if br] + [1])
    meta = dict(branches=branches, t_max=t_max)

    nc = _build(meta)

    in_maps = []
    for c in range(N_CORES):
        m = {
            "oh": np.ascontiguousarray(oh16[:, c * R:(c + 1) * R]),
            "rhs16": rhs16,
        }
        for name, tbl_row in (("cls", 0), ("ctx", 1)):
            br = branches[name]
            if br is None:
                continue
            m[f"xgt_{name}"] = br["comp"]["xgt"][c]
            m[f"w_{name}"] = np.ascontiguousarray(br["host"]["rhs"])
            m[f"idx_{name}"] = br["comp"]["idx"][c]
            m[f"tbl_{name}"] = np.ascontiguousarray(
                np.tile(st[tbl_row][None, :], (P, 1)))
            if br["host"].get("use_bln"):
                m[f"bln_{name}"] = br["host"]["bln_rep"]
        in_maps.append(m)

    res = None
    for attempt in range(3):
        try:
            res = run_bass_kernel_spmd(nc, in_maps, core_ids=list(range(N_CORES)))
            break
        except Exception:
            # transient device errors (e.g. NRT unit-unrecoverable) usually
            # clear after a pause; rebuild the program so no stale executable
            # state is reused
            if attempt == 2:
                raise
            time.sleep(10)
            nc = _build(meta)
    _LAST["results"] = res
    _LAST["meta"] = meta

    out = np.concatenate(
        [res.results[c]["out"].reshape(B // N_CORES, S, D) for c in range(N_CORES)],
        axis=0)
    return np.ascontiguousarray(out.astype(np.float32))



# revision 10
# speedup vs baseline: 16.6951x; 16.6951x over previous
"""Trainium2 Bass kernel for nn_ContextEmbedding (embedding lookup + masked MLP branches).

Strategy (data-parallel over 8 cores, batch-sharded). Only tokens in
[SPECIAL_OFFSET, SPECIAL_OFFSET+8) produce nonzero output rows (~10.5% of
rows); ExternalOutput buffers are zero-initialized by the runtime (both the
native run_neff path and the PJRT/axon path donate pre-zeroed buffers), so the
kernel only materializes the nonzero rows:

  1. Host compacts the special rows of each core's shard into a padded
     [OTHER | CLS | CTX] slot list (sizes = max over cores, so the SPMD
     program is shared; per-core slot *values* are inputs).
  2. Device gathers table rows for the OTHER slots straight from HBM into
     SBUF (dma_gather, fp16, <=1024 indices per instruction).
  3. Device runs the CLS/CTX branch MLP -> LayerNorm -> ReLU on the
     compacted feature columns (fp16 matmul into f32 PSUM; Square-accum +
     Rsqrt + scaled ReLU keep the whole LN on one ACT table set) and adds
     the host-replicated CLS/CTX table row on DVE, writing those slot
     blocks directly.
  4. Device scatter-adds every slot into the fp16 output at its row index
     (dma_scatter_add; padding slots land on trash rows past the real
     output, which the host slices off).

Host post-processing is only reshaping: slice trash rows, upcast fp16->f32,
concatenate the 8 shards.
"""

import time
import numpy as np

import concourse.bass as bass  # noqa: F401  (kept for parity with utils)
import concourse.bacc as bacc
import concourse.mybir as mybir
from concourse.tile import TileContext
from concourse.bass_utils import run_bass_kernel_spmd

N_CORES = 8
B, S, D = 256, 512, 256
NUM_CONTEXT = 16
NUM_SPECIAL = 8
SPECIAL_OFFSET = 68  # 52 cards + 16 bet bins
CLS_TOK = SPECIAL_OFFSET + 0
CTX_TOK = SPECIAL_OFFSET + 1
LN_EPS = 1e-5
P = 128
R = (B * S) // N_CORES       # rows per core
TRASH = 128                  # extra output rows that absorb padding-slot writes
SWDGE_CAP = 1024             # max indices per gather/scatter instruction

_LAST = {}


def _branch_host(W, bvec, g, b_ln):
    """Host-side prep of one MLP branch: center the linear layer so the LN mean
    subtraction folds into the weights, and (when LN gamma is uniform) fold
    gamma in too. Returns the rhs matrix for the device matmul plus the scale
    constant for the rsqrt(var+eps) activation."""
    W64 = np.asarray(W, np.float64)
    b64 = np.asarray(bvec, np.float64)
    g64 = np.asarray(g, np.float64)
    bln64 = np.asarray(b_ln, np.float64)
    wm = W64.mean(axis=1, keepdims=True)
    bm = b64.mean()
    Wc = W64 - wm
    bc = b64 - bm
    g_uniform = bool(np.all(g64 == g64.flat[0]))
    use_bln = bool(np.any(bln64 != 0.0))
    if g_uniform and not use_bln:
        gv = float(g64.flat[0])
        if gv == 0.0:
            return dict(mode="zero")
        rhs = np.concatenate([Wc * gv, (bc * gv)[None, :]], axis=0)  # [K+1, D]
        return dict(mode="fast", rhs=rhs.astype(np.float32),
                    sqrt_scale=float(1.0 / (D * gv * gv)))
    # general: rhs = [hc block | hg block]; hc drives the variance, hg the output
    Wg = Wc * g64[None, :]
    bg = bc * g64
    rhs = np.concatenate(
        [np.concatenate([Wc, bc[None, :]], axis=0),
         np.concatenate([Wg, bg[None, :]], axis=0)], axis=1)  # [K+1, 2D]
    return dict(mode="general", rhs=rhs.astype(np.float32),
                sqrt_scale=float(1.0 / D), use_bln=use_bln,
                bln_rep=np.tile(bln64.astype(np.float32)[None, :], (P, 1)))


def _wrap16(vals):
    """SWDGE index layout: entry i lives at [i % 16, i // 16], replicated to
    all 8 groups of 16 partitions (one per Q7 core)."""
    n = len(vals)
    assert n % 16 == 0
    a = np.asarray(vals, np.int16).reshape(n // 16, 16).T  # [16, n/16]
    return np.ascontiguousarray(np.tile(a, (8, 1)))        # [128, n/16]


def _chunks(lo, hi):
    """Split the slot range [lo, hi) into SWDGE_CAP-sized chunks."""
    out = []
    while lo < hi:
        n = min(SWDGE_CAP, hi - lo)
        out.append((lo, n))
        lo += n
    return out


def _build(meta):
    nc = bacc.Bacc(None)
    f32 = mybir.dt.float32
    f16 = mybir.dt.float16
    i16 = mybir.dt.int16
    Relu = mybir.ActivationFunctionType.Relu
    Sqrt = mybir.ActivationFunctionType.Sqrt
    Square = mybir.ActivationFunctionType.Square

    N = meta["N"]            # total padded slots (multiple of 128)
    PO = meta["PO"]          # leading OTHER-section slots
    NBLK = N // P
    NI = N // 16
    KB, CB = meta["kblob"]   # weights blob rows/cols
    CR = meta["repcols"]     # rep blob cols

    tbl_d = nc.dram_tensor("tbl", [NUM_SPECIAL, D], f16, kind="ExternalInput")
    idx_d = nc.dram_tensor("idx", [P, 2 * NI], i16, kind="ExternalInput")
    blob_d = nc.dram_tensor("blob", [KB, CB], f16, kind="ExternalInput")
    rep_d = nc.dram_tensor("rep", [P, CR], f16, kind="ExternalInput")
    bln_handles = {}
    for name, br in meta["branches"].items():
        if br is not None and br["host"].get("use_bln"):
            bln_handles[name] = nc.dram_tensor(
                f"bln_{name}", [P, D], f32, kind="ExternalInput")
    out_d = nc.dram_tensor("out", [R + TRASH, D], f16, kind="ExternalOutput")

    with TileContext(nc) as tc:
        with (
            tc.tile_pool(name="const", bufs=1) as cpool,
            tc.tile_pool(name="work", bufs=6) as wpool,
            tc.tile_pool(name="ps", bufs=2, space="PSUM") as pspool,
        ):
            idx_sb = cpool.tile([P, 2 * NI], i16, tag="idx")
            nc.sync.dma_start(out=idx_sb[:], in_=idx_d[:])
            blob_sb = cpool.tile([KB, CB], f16, tag="blob")
            nc.scalar.dma_start(out=blob_sb[:], in_=blob_d[:])
            rep_sb = cpool.tile([P, CR], f16, tag="rep")
            nc.sync.dma_start(out=rep_sb[:], in_=rep_d[:])
            eps_sb = cpool.tile([P, 1], f32, tag="eps")
            nc.vector.memset(eps_sb[:], LN_EPS)
            # Tiny Sqrt first so the ACT table pass picks the set holding
            # Sqrt+Square+Relu up front -- one table load instead of two.
            warm = wpool.tile([P, 1], f32, tag="warm")
            nc.scalar.activation(out=warm[:], in_=eps_sb[:], func=Sqrt)
            bln_sb = {}
            for name, h in bln_handles.items():
                bln_sb[name] = cpool.tile([P, D], f32, tag=f"bln_{name}",
                                          name=f"bln_{name}_sb")
                nc.sync.dma_start(out=bln_sb[name][:], in_=h[:])

            gt = cpool.tile([P, NBLK * D], f16, tag="gt")

            # Gather table rows for the OTHER slots (ids 2..7 plus pads).
            for off, n in _chunks(0, PO):
                nc.gpsimd.dma_gather(
                    gt[:, off // P * D:(off + n) // P * D]
                    .rearrange("p (b e) -> p b e", e=D),
                    tbl_d[:],
                    idx_sb[:, off // 16:(off + n) // 16],
                    n, n, D,
                )

            # Branch pipelines: slot i of a section maps to partition i%128 of
            # block i//128, matching the scatter slot layout. The LN math uses
            # one ACT table set (Square/Rsqrt/Relu) -> a single table load.
            for name in ("cls", "ctx"):
                br = meta["branches"].get(name)
                if br is None:
                    continue
                host = br["host"]
                nb = br["nslots"] // P
                c_x, c_w, c_rep = br["cols"]
                K1, ND = host["rhs"].shape
                for b in range(nb):
                    psA = pspool.tile([P, ND], f32, tag=f"psA_{name}",
                                      name=f"psA_{name}_{b}")
                    nc.tensor.matmul(
                        out=psA[:],
                        lhsT=blob_sb[0:KB, c_x + b * P:c_x + (b + 1) * P],
                        rhs=blob_sb[0:KB, c_w:c_w + ND],
                        start=True, stop=True)
                    sq = wpool.tile([P, D], f32, tag="sq")
                    ss = wpool.tile([P, 1], f32, tag="ss", name=f"ss_{name}_{b}")
                    nc.scalar.activation(out=sq[:], in_=psA[:, 0:D],
                                         func=Square, accum_out=ss[:])
                    sd = wpool.tile([P, 1], f32, tag="sd", name=f"sd_{name}_{b}")
                    nc.scalar.activation(out=sd[:], in_=ss[:], func=Sqrt,
                                         bias=eps_sb[:, 0:1],
                                         scale=host["sqrt_scale"])
                    rstd = wpool.tile([P, 1], f32, tag="rstd",
                                      name=f"rstd_{name}_{b}")
                    nc.vector.reciprocal(out=rstd[:], in_=sd[:])
                    rr = wpool.tile([P, D], f16, tag="rr", name=f"rr_{name}_{b}")
                    if host["mode"] == "fast":
                        nc.scalar.activation(out=rr[:], in_=psA[:, 0:D],
                                             func=Relu, scale=rstd[:, 0:1])
                    else:
                        pre = wpool.tile([P, D], f32, tag="pre")
                        nc.vector.tensor_scalar_mul(out=pre[:],
                                                    in0=psA[:, D:2 * D],
                                                    scalar1=rstd[:, 0:1])
                        if host.get("use_bln"):
                            nc.vector.tensor_add(out=pre[:], in0=pre[:],
                                                 in1=bln_sb[name][:])
                        nc.scalar.activation(out=rr[:], in_=pre[:], func=Relu)
                    col = (br["blk0"] + b) * D
                    nc.vector.tensor_add(out=gt[:, col:col + D], in0=rr[:],
                                         in1=rep_sb[:, c_rep:c_rep + D])

            # Scatter every slot to its output row; padding slots hit the
            # trash rows. The pure-OTHER chunks only depend on the gather;
            # the tail chunk (OTHER remainder + CLS/CTX) also waits on the
            # branch adds. Chunk boundaries are multiples of 128.
            cuts = [off for off, _ in _chunks(0, PO)] + [N]
            spans = list(zip(cuts, cuts[1:]))
            # The tail span (OTHER remainder + CLS/CTX) is gated by the branch
            # adds, which finish before the big OTHER gather round-trips; emit
            # it first so the WAW-serialized scatter chain starts earlier.
            spans = spans[-1:] + spans[:-1]
            for off, nxt in spans:
                n = nxt - off
                for o2, n2 in _chunks(off, off + n):
                    nc.gpsimd.dma_scatter_add(
                        out_d[:],
                        gt[:, o2 // P * D:(o2 + n2) // P * D]
                        .rearrange("p (b e) -> p b e", e=D),
                        idx_sb[:, NI + o2 // 16:NI + (o2 + n2) // 16],
                        n2, n2, D,
                    )
    nc.compile()
    return nc


def kernel(**inputs):
    tok = np.asarray(inputs["token_ids"]).reshape(-1).astype(np.int64)
    x = np.asarray(inputs["context_features"], np.float32).reshape(-1, NUM_CONTEXT)
    st = np.asarray(inputs["special_table"], np.float32)

    host_br = {
        "cls": _branch_host(inputs["cls_w"], inputs["cls_b"],
                            inputs["cls_ln_g"], inputs["cls_ln_b"]),
        "ctx": _branch_host(inputs["ctx_w"], inputs["ctx_b"],
                            inputs["ctx_ln_g"], inputs["ctx_ln_b"]),
    }
    kfeat = {"cls": 3, "ctx": NUM_CONTEXT}

    rows = {name: [] for name in ("other", "cls", "ctx")}
    for c in range(N_CORES):
        tc_ = tok[c * R:(c + 1) * R]
        rows["other"].append(np.nonzero(
            (tc_ >= SPECIAL_OFFSET + 2) & (tc_ < SPECIAL_OFFSET + NUM_SPECIAL))[0])
        rows["cls"].append(np.nonzero(tc_ == CLS_TOK)[0])
        rows["ctx"].append(np.nonzero(tc_ == CTX_TOK)[0])

    def _padded(name):
        mx = max(len(r) for r in rows[name])
        return ((mx + P - 1) // P) * P if mx else 0

    sizes = {"other": _padded("other")}
    branches = {}
    for name in ("cls", "ctx"):
        n = _padded(name)
        use = n > 0 and host_br[name]["mode"] != "zero"
        sizes[name] = n if use else 0
        branches[name] = dict(host=host_br[name], nslots=sizes[name]) if use else None

    # Slot layout: [OTHER | CLS | CTX]; all section bases are multiples of 128.
    base = {}
    off = 0
    for name in ("other", "cls", "ctx"):
        base[name] = off
        off += sizes[name]
    N = off
    assert N > 0 and N % P == 0

    # Weights blob [KB, CB] fp16: per branch [xgt | rhs] column sections,
    # rows zero-padded to the max K. Rep blob [128, CR]: CLS/CTX table rows
    # replicated across partitions.
    KB = max([br["host"]["rhs"].shape[0] for br in branches.values() if br] + [1])
    CB = 0
    CR = 0
    for name in ("cls", "ctx"):
        br = branches[name]
        if br is None:
            continue
        K1, ND = br["host"]["rhs"].shape
        br["blk0"] = base[name] // P
        br["cols"] = (CB, CB + br["nslots"], CR)
        CB += br["nslots"] + ND
        CR += D
    CB = max(CB, P)
    CR = max(CR, D)

    meta = dict(N=N, PO=sizes["other"], branches=branches,
                kblob=(KB, CB), repcols=CR)
    nc = _build(meta)

    blob = np.zeros((KB, CB), np.float16)
    rep = np.zeros((P, CR), np.float16)
    for name, tbl_row in (("cls", 0), ("ctx", 1)):
        br = branches[name]
        if br is None:
            continue
        K1, ND = br["host"]["rhs"].shape
        c_x, c_w, c_rep = br["cols"]
        blob[:K1, c_w:c_w + ND] = br["host"]["rhs"].astype(np.float16)
        rep[:, c_rep:c_rep + D] = np.tile(st[tbl_row].astype(np.float16)[None, :],
                                          (P, 1))

    in_maps = []
    tbl16 = np.ascontiguousarray(st.astype(np.float16))
    for c in range(N_CORES):
        gids = np.zeros(N, np.int16)
        sidx = np.empty(N, np.int64)
        sidx[:] = R + (np.arange(N) % TRASH)          # default: trash rows
        for name in ("other", "cls", "ctx"):
            if sizes[name] == 0:
                continue
            r = rows[name][c]
            o = base[name]
            sidx[o:o + len(r)] = r
            if name == "other":
                gids[o:o + len(r)] = (tok[c * R + r] - SPECIAL_OFFSET).astype(np.int16)
        cblob = blob.copy()
        for name in ("cls", "ctx"):
            br = branches[name]
            if br is None:
                continue
            k = kfeat[name]
            r = rows[name][c]
            c_x, c_w, c_rep = br["cols"]
            if len(r):
                cblob[:k, c_x:c_x + len(r)] = x[c * R + r][:, :k].T.astype(np.float16)
            cblob[k, c_x:c_x + len(r)] = 1.0  # bias row only on real slots
        idx2 = np.concatenate([_wrap16(gids), _wrap16(sidx.astype(np.int16))], axis=1)
        m = {
            "tbl": tbl16,
            "idx": np.ascontiguousarray(idx2),
            "blob": np.ascontiguousarray(cblob),
            "rep": rep,
        }
        for name in ("cls", "ctx"):
            br = branches[name]
            if br is not None and br["host"].get("use_bln"):
                m[f"bln_{name}"] = br["host"]["bln_rep"]
        in_maps.append(m)

    res = None
    for attempt in range(3):
        try:
            res = run_bass_kernel_spmd(nc, in_maps, core_ids=list(range(N_CORES)))
            break
        except Exception:
            # transient device errors (e.g. NRT unit-unrecoverable) usually
            # clear after a pause; rebuild the program so no stale executable
            # state is reused
            if attempt == 2:
                raise
            time.sleep(10)
            nc = _build(meta)
    _LAST["results"] = res
    _LAST["meta"] = meta

    out = np.concatenate(
        [res.results[c]["out"][:R].astype(np.float32).reshape(B // N_CORES, S, D)
         for c in range(N_CORES)], axis=0)
    return np.ascontiguousarray(out)
